# revision 3
# baseline (speedup 1.0000x reference)
"""Trainium2 Bass kernel for nn_F0Resonance.

Math: out[r, s] = N(sum_{o=1..16} d_r^o * sin(o*(s+1)*W_r)), N = per-row
max-abs normalization, for 256 rows (B=4 x E=64) and S=32768 samples.

Design: write s = k*256 + b (k in [0,128), b in [0,256)). Then
  sum_o d^o sin(o(s+1)W) = sum_{o,c} stat[(o,c), k] * states[(o,c), b]
with stat[(o,0),k] = d^o sin(o*k*256*W), stat[(o,1),k] = d^o cos(o*k*256*W)
and states[(o,0),b] = cos(o*(b+1)*W), states[(o,1),b] = sin(o*(b+1)*W).
The 32-term contraction runs on the TensorEngine as one [32,128]x[32,256]
fp32 matmul per row; the PSUM tile [128,256] is exactly the row's 32768
samples, contiguous in DRAM.

Host precomputes (f64, exactly range-reduced) the stationary tables and the
state phases in centered turns [-0.5, 0.5); the device evaluates Sin (ACT
table is only valid on [-pi, pi]), does the matmuls, the abs-max reduction,
and the normalization (second matmul pass with inv-max-scaled stationary).

Sharding: pure data-parallel, 32 consecutive rows per core, 8 cores.
"""
import numpy as np
from contextlib import ExitStack

import concourse.bacc as bacc
import concourse.mybir as mybir
import concourse.tile as tile
import concourse.bass_isa as bass_isa
from concourse.bass_utils import run_bass_kernel_spmd

F32 = mybir.dt.float32

B, E, O, S = 4, 64, 16, 32768
ROWS = B * E              # 256
NCORES = 8
RPC = ROWS // NCORES      # 32 rows per core
KP, NB = 128, 256         # s = k*NB + b
GROUPS = RPC // 4         # 8 groups of 4 local rows; states partition = lr*32 + 2*(o-1) + c

MIN_FREQ = 20 / 11025
MAX_FREQ = 3000 / 11025
FREQ_RANGE = MAX_FREQ - MIN_FREQ
TWO_PI = 2 * np.pi

_PROGRAM = None


def _build_program():
    nc = bacc.Bacc("TRN2", target_bir_lowering=False, debug=False)

    stat_in = nc.dram_tensor("stat", [2 * O, RPC * KP], F32, kind="ExternalInput").ap()
    ph_in = nc.dram_tensor("stphase", [2 * O, RPC * NB], F32, kind="ExternalInput").ap()
    out_d = nc.dram_tensor("out", [RPC, KP, NB], F32, kind="ExternalOutput").ap()

    with tile.TileContext(nc) as tc, ExitStack() as ctx:
        singles = ctx.enter_context(tc.tile_pool(name="singles", bufs=1))
        psum_a = ctx.enter_context(tc.tile_pool(name="psum_a", bufs=4, space="PSUM"))
        psum_b = ctx.enter_context(tc.tile_pool(name="psum_b", bufs=4, space="PSUM"))
        outp = ctx.enter_context(tc.tile_pool(name="outp", bufs=4))
        stsp = ctx.enter_context(tc.tile_pool(name="stsp", bufs=4))

        stat = singles.tile([2 * O, RPC * KP], F32)
        states = singles.tile([2 * O, RPC * NB], F32)
        phchunks = ctx.enter_context(tc.tile_pool(name="phchunks", bufs=3))
        CH = 4  # rows per in-chunk
        for g in range(RPC // CH):
            sl = slice(g * CH * KP, (g + 1) * CH * KP)
            nc.sync.dma_start(stat[:, sl], stat_in[:, sl])
            ph = phchunks.tile([2 * O, CH * NB], F32, tag="ph")
            psl = slice(g * CH * NB, (g + 1) * CH * NB)
            nc.sync.dma_start(ph[:], ph_in[:, psl])
            # states = sin(2*pi*phase); phases in [-0.5, 0.5) -> args in [-pi, pi)
            nc.scalar.activation(states[:, psl], ph[:],
                                 mybir.ActivationFunctionType.Sin,
                                 scale=float(TWO_PI))

        mxcols = singles.tile([KP, RPC], F32)

        def rhs_of(r):
            return states[:, r * NB:(r + 1) * NB]

        def lhs_of(r):
            return stat[:, r * KP:(r + 1) * KP]

        # Phase A: unnormalized signal, per-row abs-max
        for r in range(RPC):
            psA = psum_a.tile([KP, NB], F32, tag="psA")
            nc.tensor.matmul(psA[:], lhs_of(r), rhs_of(r), start=True, stop=True)
            nc.vector.tensor_reduce(mxcols[:, r:r + 1], psA[:],
                                    mybir.AxisListType.X, mybir.AluOpType.max,
                                    apply_absolute_value=True)

        # Cross-partition max, then 1/(mx + 1e-8)
        mxall = singles.tile([KP, RPC], F32)
        nc.gpsimd.partition_all_reduce(mxall[:], mxcols[:], channels=KP,
                                       reduce_op=bass_isa.ReduceOp.absmax)
        mxeps = singles.tile([KP, RPC], F32)
        nc.vector.tensor_scalar(mxeps[:], mxall[:], 1e-8, None, mybir.AluOpType.add)
        inv = singles.tile([KP, RPC], F32)
        nc.vector.reciprocal(inv[:], mxeps[:])

        # Phase B: scaled stationary -> normalized signal -> SBUF -> DRAM
        for r in range(RPC):
            sts = stsp.tile([2 * O, KP], F32, tag="sts")
            nc.vector.tensor_scalar(sts[:], lhs_of(r),
                                    inv[0:2 * O, r:r + 1], None,
                                    mybir.AluOpType.mult)
            psB = psum_b.tile([KP, NB], F32, tag="psB")
            nc.tensor.matmul(psB[:], sts[:], rhs_of(r), start=True, stop=True)
            ot = outp.tile([KP, NB], F32, tag="ot")
            nc.scalar.copy(ot[:], psB[:])
            nc.sync.dma_start(out_d[r], ot[:])

    nc.compile()
    return nc


def _centered_frac(x):
    return x - np.round(x)


def _host_tables(f0, decay_coefficients, freq_spacing):
    """Per-row W (angular increment) and d (decay), f64; returns per-core
    (stat, stphase) arrays."""
    f0 = np.abs(f0.astype(np.float64).reshape(ROWS))
    dc = decay_coefficients.astype(np.float64).reshape(ROWS)
    fs = freq_spacing.astype(np.float64).reshape(ROWS)

    dv = 1.0 / (1.0 + np.exp(-(1.0 / (1.0 + np.exp(-dc)))))
    d = 0.01 + dv * (1.0 - 0.01) * 0.95
    W = (MIN_FREQ + f0 * FREQ_RANGE) * np.pi * fs

    o = np.arange(1, O + 1, dtype=np.float64)            # (16,)
    dpow = d[:, None] ** o[None, :]                      # (256, 16)

    k = np.arange(KP, dtype=np.float64)
    # stationary phase (exact f64 range reduction)
    thA = TWO_PI * _centered_frac((o[None, :, None] * NB / TWO_PI)
                                  * W[:, None, None] * k[None, None, :])  # (256,16,128)
    statS = dpow[:, :, None] * np.sin(thA)
    statC = dpow[:, :, None] * np.cos(thA)
    stat_rows = np.empty((ROWS, 2 * O, KP), np.float32)
    stat_rows[:, 0::2] = statS                            # c=0 pairs cos-state
    stat_rows[:, 1::2] = statC                            # c=1 pairs sin-state

    b = np.arange(1, NB + 1, dtype=np.float64)
    tb = (o[None, :, None] / TWO_PI) * W[:, None, None] * b[None, None, :]  # (256,16,256) turns
    ph_sin = _centered_frac(tb)                           # c=1
    ph_cos = _centered_frac(tb + 0.25)                    # c=0: sin(2pi t + pi/2) = cos
    ph_rows = np.empty((ROWS, 2 * O, NB), np.float32)
    ph_rows[:, 0::2] = ph_cos
    ph_rows[:, 1::2] = ph_sin

    stats, phases = [], []
    for c in range(NCORES):
        rows = slice(c * RPC, (c + 1) * RPC)
        # stat: [2O, RPC*KP], row-major in free dim
        sc = stat_rows[rows].transpose(1, 0, 2).reshape(2 * O, RPC * KP)
        stats.append(np.ascontiguousarray(sc))
        # stphase: [2O, RPC*NB]; partition = oc, free = (row, b)
        pc = ph_rows[rows].transpose(1, 0, 2).reshape(2 * O, RPC * NB)
        phases.append(np.ascontiguousarray(pc))
    return stats, phases


def _run(inputs, trace=False, **trace_kwargs):
    global _PROGRAM
    if _PROGRAM is None:
        _PROGRAM = _build_program()
    stats, phases = _host_tables(inputs["f0"], inputs["decay_coefficients"],
                                 inputs["freq_spacing"])
    in_maps = [{"stat": stats[c], "stphase": phases[c]} for c in range(NCORES)]
    res = run_bass_kernel_spmd(_PROGRAM, in_maps, core_ids=list(range(NCORES)),
                               trace=trace, **trace_kwargs)
    rows = np.concatenate([res.results[c]["out"].reshape(RPC, S)
                           for c in range(NCORES)], axis=0)
    return rows.reshape(B, E, S).astype(np.float32), res


def kernel(f0, decay_coefficients, phase_offsets, freq_spacing):
    out, _ = _run(dict(f0=np.asarray(f0), decay_coefficients=np.asarray(decay_coefficients),
                       phase_offsets=np.asarray(phase_offsets),
                       freq_spacing=np.asarray(freq_spacing)))
    return out


# revision 5
# speedup vs baseline: 1.5873x; 1.5873x over previous
"""Trainium2 Bass kernel for nn_F0Resonance.

Math: out[r, s] = N(sum_{o=1..16} d_r^o * sin(o*(s+1)*W_r)), N = per-row
max-abs normalization, for 256 rows (B=4 x E=64) and S=32768 samples.

Design: write s = k*256 + b (k in [0,128), b in [0,256)). Then
  sum_o d^o sin(o(s+1)W) = sum_{o,c} stat[(o,c), k] * states[(o,c), b]
with stat[(o,0),k] = d^o sin(o*k*256*W), stat[(o,1),k] = d^o cos(o*k*256*W)
and states[(o,0),b] = cos(o*(b+1)*W), states[(o,1),b] = sin(o*(b+1)*W).
The 32-term contraction runs on the TensorEngine as one [32,128]x[32,256]
matmul per row (operands bitcast to float32r for ~3x PE throughput at
~fp32 accuracy); the PSUM tile [128,256] is the row's 32768 samples,
contiguous in DRAM.

Host precomputes (f64, exactly range-reduced) the stationary tables and the
state phases in centered turns [-0.5, 0.5); the device evaluates Sin (ACT
table is only valid on [-pi, pi]), does the matmuls, the abs-max reduction,
and fuses normalization into the PSUM->SBUF copy (per-partition scale on
ACT/DVE). Normalization is per row, so the whole kernel streams row by row
with no global barrier.

Sharding: pure data-parallel, 32 consecutive rows per core, 8 cores.
"""
import numpy as np
from contextlib import ExitStack

import concourse.bacc as bacc
import concourse.mybir as mybir
import concourse.tile as tile
import concourse.bass_isa as bass_isa
from concourse.bass_utils import run_bass_kernel_spmd

F32 = mybir.dt.float32
F32R = mybir.dt.float32r

B, E, O, S = 4, 64, 16, 32768
ROWS = B * E              # 256
NCORES = 8
RPC = ROWS // NCORES      # 32 rows per core
KP, NB = 128, 256         # s = k*NB + b
CH = 4                    # rows per input chunk / normalization group

MIN_FREQ = 20 / 11025
MAX_FREQ = 3000 / 11025
FREQ_RANGE = MAX_FREQ - MIN_FREQ
TWO_PI = 2 * np.pi

USE_F32R = True
_PROGRAM = None


def _build_program():
    nc = bacc.Bacc("TRN2", target_bir_lowering=False, debug=False)

    stat_in = nc.dram_tensor("stat", [2 * O, RPC * KP],
                             F32R if USE_F32R else F32, kind="ExternalInput").ap()
    ph_in = nc.dram_tensor("stphase", [2 * O, RPC * NB], F32, kind="ExternalInput").ap()
    out_d = nc.dram_tensor("out", [RPC, KP, NB], F32, kind="ExternalOutput").ap()

    nchunks = RPC // CH

    with tile.TileContext(nc) as tc, ExitStack() as ctx:
        statp = ctx.enter_context(tc.tile_pool(name="statp", bufs=nchunks))
        phasep = ctx.enter_context(tc.tile_pool(name="phasep", bufs=3))
        statesp = ctx.enter_context(tc.tile_pool(name="statesp", bufs=nchunks))
        psum = ctx.enter_context(tc.tile_pool(name="psum", bufs=6, space="PSUM"))
        outp = ctx.enter_context(tc.tile_pool(name="outp", bufs=8))
        mxp = ctx.enter_context(tc.tile_pool(name="mxp", bufs=4))

        stat_t, states_t = [], []
        for g in range(nchunks):
            st = statp.tile([2 * O, CH * KP], F32R if USE_F32R else F32, tag="stat")
            nc.sync.dma_start(st[:], stat_in[:, g * CH * KP:(g + 1) * CH * KP])
            ph = phasep.tile([2 * O, CH * NB], F32, tag="ph")
            nc.scalar.dma_start(ph[:], ph_in[:, g * CH * NB:(g + 1) * CH * NB])
            sts = statesp.tile([2 * O, CH * NB], F32R if USE_F32R else F32, tag="states")
            # states = sin(2*pi*phase); phases in [-0.5, 0.5) -> args in [-pi, pi)
            nc.scalar.activation(sts[:], ph[:], mybir.ActivationFunctionType.Sin,
                                 scale=float(TWO_PI))
            stat_t.append(st)
            states_t.append(sts)

        def lhs_of(r):
            g, lr = divmod(r, CH)
            return stat_t[g][:, lr * KP:(lr + 1) * KP]

        def rhs_of(r):
            g, lr = divmod(r, CH)
            return states_t[g][:, lr * NB:(lr + 1) * NB]

        # Stream per normalization group of CH=4 rows (2 PSUM banks).
        for q in range(RPC // CH):
            pps = []
            mx = mxp.tile([KP, CH], F32, tag="mx")
            for j in range(CH // 2):
                r0 = q * CH + 2 * j
                pp = psum.tile([KP, 2 * NB], F32, tag="pp")
                nc.tensor.matmul(pp[:, 0:NB], lhs_of(r0), rhs_of(r0),
                                 start=True, stop=True)
                nc.tensor.matmul(pp[:, NB:2 * NB], lhs_of(r0 + 1), rhs_of(r0 + 1),
                                 start=True, stop=True)
                nc.vector.tensor_reduce(mx[:, 2 * j:2 * j + 2],
                                        pp[:].rearrange("p (r b) -> p r b", r=2),
                                        mybir.AxisListType.X, mybir.AluOpType.max,
                                        apply_absolute_value=True)
                pps.append(pp)
            mxa = mxp.tile([KP, CH], F32, tag="mxa")
            nc.gpsimd.partition_all_reduce(mxa[:], mx[:], channels=KP,
                                           reduce_op=bass_isa.ReduceOp.absmax)
            inv = mxp.tile([KP, CH], F32, tag="inv")
            nc.vector.tensor_scalar(mxa[:], mxa[:], 1e-8, None, mybir.AluOpType.add)
            nc.vector.reciprocal(inv[:], mxa[:])

            for lr in range(CH):
                r = q * CH + lr
                src = pps[lr // 2][:, (lr % 2) * NB:(lr % 2 + 1) * NB]
                ot = outp.tile([KP, NB], F32, tag="ot")
                if lr % 2 == 0:
                    # ACT: fused PSUM->SBUF copy with per-partition scale
                    nc.scalar.mul(ot[:], src, inv[:, lr:lr + 1])
                else:
                    nc.vector.tensor_scalar(ot[:], src, inv[:, lr:lr + 1], None,
                                            mybir.AluOpType.mult)
                eng = nc.sync if r % 2 == 0 else nc.scalar
                eng.dma_start(out_d[r], ot[:])

    nc.compile()
    return nc


def _centered_frac(x):
    return x - np.round(x)


def _host_tables(f0, decay_coefficients, freq_spacing):
    """Per-row W (angular increment) and d (decay), f64; returns per-core
    (stat, stphase) arrays."""
    f0 = np.abs(f0.astype(np.float64).reshape(ROWS))
    dc = decay_coefficients.astype(np.float64).reshape(ROWS)
    fs = freq_spacing.astype(np.float64).reshape(ROWS)

    dv = 1.0 / (1.0 + np.exp(-(1.0 / (1.0 + np.exp(-dc)))))
    d = 0.01 + dv * (1.0 - 0.01) * 0.95
    W = (MIN_FREQ + f0 * FREQ_RANGE) * np.pi * fs

    o = np.arange(1, O + 1, dtype=np.float64)            # (16,)
    dpow = d[:, None] ** o[None, :]                      # (256, 16)

    k = np.arange(KP, dtype=np.float64)
    # stationary phase (exact f64 range reduction)
    thA = TWO_PI * _centered_frac((o[None, :, None] * NB / TWO_PI)
                                  * W[:, None, None] * k[None, None, :])  # (256,16,128)
    statS = dpow[:, :, None] * np.sin(thA)
    statC = dpow[:, :, None] * np.cos(thA)
    stat_rows = np.empty((ROWS, 2 * O, KP), np.float32)
    stat_rows[:, 0::2] = statS                            # c=0 pairs cos-state
    stat_rows[:, 1::2] = statC                            # c=1 pairs sin-state

    b = np.arange(1, NB + 1, dtype=np.float64)
    tb = (o[None, :, None] / TWO_PI) * W[:, None, None] * b[None, None, :]  # (256,16,256) turns
    ph_sin = _centered_frac(tb)                           # c=1
    ph_cos = _centered_frac(tb + 0.25)                    # c=0: sin(2pi t + pi/2) = cos
    ph_rows = np.empty((ROWS, 2 * O, NB), np.float32)
    ph_rows[:, 0::2] = ph_cos
    ph_rows[:, 1::2] = ph_sin

    stats, phases = [], []
    for c in range(NCORES):
        rows = slice(c * RPC, (c + 1) * RPC)
        sc = stat_rows[rows].transpose(1, 0, 2).reshape(2 * O, RPC * KP)
        stats.append(np.ascontiguousarray(sc))
        pc = ph_rows[rows].transpose(1, 0, 2).reshape(2 * O, RPC * NB)
        phases.append(np.ascontiguousarray(pc))
    return stats, phases


def _run(inputs, trace=False, **trace_kwargs):
    global _PROGRAM
    if _PROGRAM is None:
        _PROGRAM = _build_program()
    stats, phases = _host_tables(inputs["f0"], inputs["decay_coefficients"],
                                 inputs["freq_spacing"])
    in_maps = [{"stat": stats[c], "stphase": phases[c]} for c in range(NCORES)]
    res = run_bass_kernel_spmd(_PROGRAM, in_maps, core_ids=list(range(NCORES)),
                               trace=trace, **trace_kwargs)
    rows = np.concatenate([res.results[c]["out"].reshape(RPC, S)
                           for c in range(NCORES)], axis=0)
    return rows.reshape(B, E, S).astype(np.float32), res


def kernel(f0, decay_coefficients, phase_offsets, freq_spacing):
    out, _ = _run(dict(f0=np.asarray(f0), decay_coefficients=np.asarray(decay_coefficients),
                       phase_offsets=np.asarray(phase_offsets),
                       freq_spacing=np.asarray(freq_spacing)))
    return out


# revision 7
# speedup vs baseline: 1.6395x; 1.0329x over previous
"""Trainium2 Bass kernel for nn_F0Resonance.

Math: out[r, s] = N(sum_{o=1..16} d_r^o * sin(o*(s+1)*W_r)), N = per-row
max-abs normalization, for 256 rows (B=4 x E=64) and S=32768 samples.

Design: write s = k*256 + b (k in [0,128), b in [0,256)). Then
  sum_o d^o sin(o(s+1)W) = sum_{o,c} stat[(o,c), k] * states[(o,c), b]
with stat[(o,0),k] = d^o sin(o*k*256*W), stat[(o,1),k] = d^o cos(o*k*256*W)
and states[(o,0),b] = cos(o*(b+1)*W), states[(o,1),b] = sin(o*(b+1)*W).
The 32-term contraction runs on the TensorEngine as one [32,128]x[32,256]
matmul per row (operands in float32r for ~3x PE throughput at ~fp32
accuracy); the PSUM tile [128,256] is the row's 32768 samples, contiguous
in DRAM.

Host precomputes (f64, exactly range-reduced) the stationary tables and the
state phases in centered turns [-0.5, 0.5); the device evaluates Sin (ACT
table is only valid on [-pi, pi]), does the matmuls, the abs-max reduction,
and fuses normalization into the PSUM->SBUF copy (per-partition scale on
ACT/DVE). Normalization is per row, so the whole kernel streams row by row
with no global barrier.

Sharding: pure data-parallel, 32 consecutive rows per core, 8 cores.
"""
import numpy as np
from contextlib import ExitStack

import concourse.bacc as bacc
import concourse.mybir as mybir
import concourse.tile as tile
import concourse.bass_isa as bass_isa
from concourse.bass_utils import run_bass_kernel_spmd

F32 = mybir.dt.float32
F32R = mybir.dt.float32r

B, E, O, S = 4, 64, 16, 32768
ROWS = B * E              # 256
NCORES = 8
RPC = ROWS // NCORES      # 32 rows per core
KP, NB = 128, 256         # s = k*NB + b
CH = 4                    # rows per input chunk / normalization group

MIN_FREQ = 20 / 11025
MAX_FREQ = 3000 / 11025
FREQ_RANGE = MAX_FREQ - MIN_FREQ
TWO_PI = 2 * np.pi

USE_F32R = True
_PROGRAM = None


def _build_program():
    nc = bacc.Bacc("TRN2", target_bir_lowering=False, debug=False)

    stat_in = nc.dram_tensor("stat", [2 * O, RPC * KP],
                             F32R if USE_F32R else F32, kind="ExternalInput").ap()
    ph_in = nc.dram_tensor("stphase", [2 * O, RPC * NB], F32, kind="ExternalInput").ap()
    out_d = nc.dram_tensor("out", [RPC, KP, NB], F32, kind="ExternalOutput").ap()

    nchunks = RPC // CH

    with tile.TileContext(nc) as tc, ExitStack() as ctx:
        statp = ctx.enter_context(tc.tile_pool(name="statp", bufs=nchunks))
        phasep = ctx.enter_context(tc.tile_pool(name="phasep", bufs=3))
        statesp = ctx.enter_context(tc.tile_pool(name="statesp", bufs=nchunks))
        psum = ctx.enter_context(tc.tile_pool(name="psum", bufs=6, space="PSUM"))
        outp = ctx.enter_context(tc.tile_pool(name="outp", bufs=8))
        mxp = ctx.enter_context(tc.tile_pool(name="mxp", bufs=4))

        stat_t, states_t = [], []
        for g in range(nchunks):
            st = statp.tile([2 * O, CH * KP], F32R if USE_F32R else F32, tag="stat")
            nc.sync.dma_start(st[:], stat_in[:, g * CH * KP:(g + 1) * CH * KP])
            ph = phasep.tile([2 * O, CH * NB], F32, tag="ph")
            nc.scalar.dma_start(ph[:], ph_in[:, g * CH * NB:(g + 1) * CH * NB])
            sts = statesp.tile([2 * O, CH * NB], F32R if USE_F32R else F32, tag="states")
            # states = sin(2*pi*phase); phases in [-0.5, 0.5) -> args in [-pi, pi)
            nc.scalar.activation(sts[:], ph[:], mybir.ActivationFunctionType.Sin,
                                 scale=float(TWO_PI))
            stat_t.append(st)
            states_t.append(sts)

        def lhs_of(r):
            g, lr = divmod(r, CH)
            return stat_t[g][:, lr * KP:(lr + 1) * KP]

        def rhs_of(r):
            g, lr = divmod(r, CH)
            return states_t[g][:, lr * NB:(lr + 1) * NB]

        # Stream per normalization group of CH=4 rows (2 PSUM banks).
        for q in range(RPC // CH):
            pps = []
            mx = mxp.tile([KP, CH], F32, tag="mx")
            for j in range(CH // 2):
                r0 = q * CH + 2 * j
                pp = psum.tile([KP, 2 * NB], F32, tag="pp")
                nc.tensor.matmul(pp[:, 0:NB], lhs_of(r0), rhs_of(r0),
                                 start=True, stop=True)
                nc.tensor.matmul(pp[:, NB:2 * NB], lhs_of(r0 + 1), rhs_of(r0 + 1),
                                 start=True, stop=True)
                nc.vector.tensor_reduce(mx[:, 2 * j:2 * j + 2],
                                        pp[:].rearrange("p (r b) -> p r b", r=2),
                                        mybir.AxisListType.X, mybir.AluOpType.max,
                                        apply_absolute_value=True)
                pps.append(pp)
            mxa = mxp.tile([KP, CH], F32, tag="mxa")
            nc.gpsimd.partition_all_reduce(mxa[:], mx[:], channels=KP,
                                           reduce_op=bass_isa.ReduceOp.absmax)
            inv = mxp.tile([KP, CH], F32, tag="inv")
            nc.vector.tensor_scalar(mxa[:], mxa[:], 1e-8, None, mybir.AluOpType.add)
            nc.vector.reciprocal(inv[:], mxa[:])

            for lr in range(CH):
                r = q * CH + lr
                src = pps[lr // 2][:, (lr % 2) * NB:(lr % 2 + 1) * NB]
                ot = outp.tile([KP, NB], F32, tag="ot")
                if lr % 2 == 0:
                    # ACT: fused PSUM->SBUF copy with per-partition scale
                    nc.scalar.mul(ot[:], src, inv[:, lr:lr + 1])
                else:
                    nc.vector.tensor_scalar(ot[:], src, inv[:, lr:lr + 1], None,
                                            mybir.AluOpType.mult)
                eng = nc.sync if r % 2 == 0 else nc.scalar
                eng.dma_start(out_d[r], ot[:])

    nc.compile()
    return nc


def _centered_frac(x):
    return x - np.round(x)


def _host_tables(f0, decay_coefficients, freq_spacing):
    """Per-row W (angular increment) and d (decay), f64; returns per-core
    (stat, stphase) arrays."""
    f0 = np.abs(f0.astype(np.float64).reshape(ROWS))
    dc = decay_coefficients.astype(np.float64).reshape(ROWS)
    fs = freq_spacing.astype(np.float64).reshape(ROWS)

    dv = 1.0 / (1.0 + np.exp(-(1.0 / (1.0 + np.exp(-dc)))))
    d = 0.01 + dv * (1.0 - 0.01) * 0.95
    W = (MIN_FREQ + f0 * FREQ_RANGE) * np.pi * fs

    o = np.arange(1, O + 1, dtype=np.float64)            # (16,)
    dpow = d[:, None] ** o[None, :]                      # (256, 16)

    k = np.arange(KP, dtype=np.float64)
    # stationary phase (exact f64 range reduction)
    thA = TWO_PI * _centered_frac((o[None, :, None] * NB / TWO_PI)
                                  * W[:, None, None] * k[None, None, :])  # (256,16,128)
    statS = dpow[:, :, None] * np.sin(thA)
    statC = dpow[:, :, None] * np.cos(thA)
    stat_rows = np.empty((ROWS, 2 * O, KP), np.float32)
    stat_rows[:, 0::2] = statS                            # c=0 pairs cos-state
    stat_rows[:, 1::2] = statC                            # c=1 pairs sin-state

    b = np.arange(1, NB + 1, dtype=np.float64)
    tb = (o[None, :, None] / TWO_PI) * W[:, None, None] * b[None, None, :]  # (256,16,256) turns
    ph_sin = _centered_frac(tb)                           # c=1
    ph_cos = _centered_frac(tb + 0.25)                    # c=0: sin(2pi t + pi/2) = cos
    ph_rows = np.empty((ROWS, 2 * O, NB), np.float32)
    ph_rows[:, 0::2] = ph_cos
    ph_rows[:, 1::2] = ph_sin

    stats, phases = [], []
    for c in range(NCORES):
        rows = slice(c * RPC, (c + 1) * RPC)
        sc = stat_rows[rows].transpose(1, 0, 2).reshape(2 * O, RPC * KP)
        stats.append(np.ascontiguousarray(sc))
        pc = ph_rows[rows].transpose(1, 0, 2).reshape(2 * O, RPC * NB)
        phases.append(np.ascontiguousarray(pc))
    return stats, phases


def _run(inputs, trace=False, **trace_kwargs):
    global _PROGRAM
    if _PROGRAM is None:
        _PROGRAM = _build_program()
    stats, phases = _host_tables(inputs["f0"], inputs["decay_coefficients"],
                                 inputs["freq_spacing"])
    in_maps = [{"stat": stats[c], "stphase": phases[c]} for c in range(NCORES)]
    res = run_bass_kernel_spmd(_PROGRAM, in_maps, core_ids=list(range(NCORES)),
                               trace=trace, **trace_kwargs)
    rows = np.concatenate([res.results[c]["out"].reshape(RPC, S)
                           for c in range(NCORES)], axis=0)
    return rows.reshape(B, E, S).astype(np.float32), res


def kernel(f0, decay_coefficients, phase_offsets, freq_spacing):
    out, _ = _run(dict(f0=np.asarray(f0), decay_coefficients=np.asarray(decay_coefficients),
                       phase_offsets=np.asarray(phase_offsets),
                       freq_spacing=np.asarray(freq_spacing)))
    return out
